# revision 2
# baseline (speedup 1.0000x reference)
"""Trainium2 Bass kernel v2 for GQA attention (B=2,T=2048,D=2048,H=16,G=4,K=128)
with QK RMS-norm, RoPE, segment-aware causal masking, sigmoid gating, o_proj.

Sharding: 8 cores = (batch b, kv-group g); core c -> b=c//4, g=c%4.
Each core computes its 4 q-heads + 1 kv head end-to-end and a partial
o_proj ([T,D]); the host sums the 4 partials per batch (TP unshard).

v2: q/k are projected directly transposed (lhsT = weight cols, rhs = hidT),
so no DMA transposes; RMS-norm sums come from ones-matmuls; inv-rms(k)
folds into the exp's per-partition scale AP; inv-rms(q) is produced
pre-broadcast by an all-ones stationary matmul. QK is run-sliced to the
contiguous non-empty i-range. The attention-output transpose (agT, o_proj
lhsT) uses batched DVE StreamTranspose. Emission interleaves
proj(c) | attention(c-1) | o_proj(c-2) per 512-chunk to keep PE dense.
"""
import sys

sys.path.insert(0, "/opt/trn_rl_repo")

import numpy as np
import ml_dtypes

import concourse.bass as bass
import concourse.mybir as mybir
import concourse.tile as tile
from concourse.bass_utils import run_bass_kernel_spmd

FP32 = mybir.dt.float32
BF16 = mybir.dt.bfloat16
AF = mybir.ActivationFunctionType
ALU = mybir.AluOpType
BF = ml_dtypes.bfloat16

B, T, D = 2, 2048, 2048
H, G, K = 16, 4, 128
HPC = H // G              # q-heads per core = 4
EPS = 1e-6
SCALE = K ** -0.5
NT = T // 128             # 16 t-tiles
NC = 4                    # 512-chunks
NDC = D // 128            # 16 d-chunks
EMPTY, FULL, PARTIAL = 0, 1, 2


def split_multiwaits(nc):
    """This container's walrus accepts one sync-wait per instruction; hoist
    extras into standalone single-wait EventSemaphore instructions."""
    n = 0
    for fn in nc.m.functions:
        for bb in fn.blocks:
            out = []
            for ins in bb.instructions:
                si = ins.sync_info
                if si is not None and si.on_wait and len(si.on_wait) > 1:
                    waits = list(si.on_wait)
                    for w in waits[:-1]:
                        n += 1
                        out.append(mybir.InstEventSemaphore(
                            name=f"{ins.name}-w{n}", engine=ins.engine,
                            ins=[], outs=[],
                            sync_info=mybir.SyncInfo(on_wait=[w], on_update=[])))
                    ins.sync_info = mybir.SyncInfo(
                        on_wait=[waits[-1]], on_update=list(si.on_update or []))
                out.append(ins)
            bb.instructions = out
    return n


def _classify(allowed):
    cls = np.zeros((NT, NT), np.int32)
    a4 = allowed.reshape(NT, 128, NT, 128)
    any_ = a4.any(axis=(1, 3))
    all_ = a4.all(axis=(1, 3))
    cls[any_ & all_] = FULL
    cls[any_ & ~all_] = PARTIAL
    return cls


def _build_schedule(segment_ids, position_ids):
    """Union schedule across batches (SPMD: one program for all cores) plus
    per-batch mask tiles for partial (i,j)."""
    allowed = []
    for b in range(B):
        pos = position_ids[b].astype(np.int64)
        seg = segment_ids[b].astype(np.int64)
        al = (pos[:, None] >= pos[None, :]) & (seg[:, None] == seg[None, :])
        allowed.append(al)
    cls_b = [_classify(al) for al in allowed]
    cls = np.maximum(cls_b[0], cls_b[1])  # EMPTY only if both empty
    cls[(cls_b[0] == FULL) & (cls_b[1] == PARTIAL)] = PARTIAL
    cls[(cls_b[1] == FULL) & (cls_b[0] == PARTIAL)] = PARTIAL

    mask_idx = {}
    masksT = [[], []]   # per batch: list of [128s,128t] f32 (deduped pairs)
    seen = {}
    for i in range(NT):
        for j in range(NT):
            if cls[i, j] == PARTIAL:
                subs = [allowed[b][i * 128:(i + 1) * 128, j * 128:(j + 1) * 128].T
                        for b in range(B)]
                key = subs[0].tobytes() + subs[1].tobytes()
                if key not in seen:
                    seen[key] = len(masksT[0])
                    for b in range(B):
                        masksT[b].append(subs[b].astype(np.float32))
                mask_idx[(i, j)] = seen[key]

    # per chunk: list of (j, r0, r1): contiguous run of non-empty i = 4c+r
    runs = []
    for c in range(NC):
        lst = []
        for j in range(NT):
            rs = [r for r in range(4) if cls[c * 4 + r, j] != EMPTY]
            if rs:
                assert rs == list(range(rs[0], rs[-1] + 1)), (c, j, rs)
                lst.append((j, rs[0], rs[-1]))
        runs.append(lst)
    pv = {i: [j for j in range(NT) if cls[i, j] != EMPTY] for i in range(NT)}
    return cls, mask_idx, masksT, runs, pv


def _build_program(n_masks, cls, mask_idx, runs, pv):
    nc = bass.Bass()
    P = 128
    hidT = nc.declare_dram_parameter("hidT", [D, T], BF16, isOutput=False)
    # wq cols per dc-block: [q_h0|q_h1|q_h2|q_h3|gate_h0..h3 (512)]
    wq = nc.declare_dram_parameter("wq", [D, HPC * 2 * K], BF16, isOutput=False)
    wk = nc.declare_dram_parameter("wk", [D, K], BF16, isOutput=False)
    wv = nc.declare_dram_parameter("wv", [D, K], BF16, isOutput=False)
    wo = nc.declare_dram_parameter("wo", [HPC * K, D], BF16, isOutput=False)
    # transposed rope tables [K, T]; q-tables have SCALE folded in
    cq = nc.declare_dram_parameter("cq", [K, T], BF16, isOutput=False)
    sq_ = nc.declare_dram_parameter("sq", [K, T], BF16, isOutput=False)
    ck = nc.declare_dram_parameter("ck", [K, T], BF16, isOutput=False)
    sk = nc.declare_dram_parameter("sk", [K, T], BF16, isOutput=False)
    masks = nc.declare_dram_parameter("masks", [max(n_masks, 1), 128, 128], BF16,
                                      isOutput=False)
    out = nc.declare_dram_parameter("out", [T, D], BF16, isOutput=True)

    maxrun = max(len(r) for r in runs)
    with tile.TileContext(nc) as tc:
        with tc.tile_pool(name="res", bufs=1) as res, \
             tc.tile_pool(name="hid", bufs=8) as hid, \
             tc.tile_pool(name="acc", bufs=3, space="PSUM") as accp, \
             tc.tile_pool(name="big", bufs=5, space="PSUM") as bigp, \
             tc.tile_pool(name="stage", bufs=3) as stage, \
             tc.tile_pool(name="pt", bufs=maxrun + 2) as ptpool, \
             tc.tile_pool(name="agp", bufs=2) as agp, \
             tc.tile_pool(name="oop", bufs=2) as oop:

            # ---- resident loads, woven across queues so PE starts fast ----
            wq_sb = res.tile([P, NDC * 1024], BF16, tag="wq")
            wk_sb = res.tile([P, NDC * 128], BF16, tag="wk")
            wv_sb = res.tile([P, NDC * 128], BF16, tag="wv")
            wo_sb = res.tile([P, HPC * D], BF16, tag="wo")
            ht_q = {}   # (chunk, quarter) -> tile [P, 4dc * 512]
            hsrc = hidT[:].rearrange("(c p) t -> p c t", p=P)

            def load_ht_q(c, q, eng):
                t_ = hid.tile([P, 4 * 512], BF16, tag="ht")
                eng.dma_start(
                    out=t_[:].rearrange("p (a j) -> p a j", a=4),
                    in_=hsrc[:, 4 * q:4 * q + 4, c * 512:(c + 1) * 512])
                ht_q[(c, q)] = t_

            def load_wq(dc, eng):
                eng.dma_start(out=wq_sb[:, dc * 1024:(dc + 1) * 1024],
                              in_=wq[dc * P:(dc + 1) * P, :])

            # Prologue: stripe loads round-robin over the 3 DMA queues in
            # the exact order region-0's dc-major qT matmuls consume them
            # (~110 GB/s per queue, ~330 aggregate). kTvT runs after qT.
            rr = [nc.sync, nc.gpsimd, nc.scalar]
            rrn = [0]

            def rrload(fn):
                fn(rr[rrn[0] % 3])
                rrn[0] += 1

            rrload(lambda e: load_ht_q(0, 0, e))
            for dc in range(NDC):
                rrload(lambda e, dc=dc: load_wq(dc, e))
                if dc % 4 == 3 and dc < 12:
                    rrload(lambda e, q=dc // 4 + 1: load_ht_q(0, q, e))
            rrload(lambda e: e.dma_start(
                out=wk_sb[:].rearrange("p (c j) -> p c j", c=NDC),
                in_=wk[:].rearrange("(c p) j -> p c j", p=P)))
            rrload(lambda e: e.dma_start(
                out=wv_sb[:].rearrange("p (c j) -> p c j", c=NDC),
                in_=wv[:].rearrange("(c p) j -> p c j", p=P)))
            tabs = {}
            for nm, t_ in (("cq", cq), ("sq", sq_), ("ck", ck), ("sk", sk)):
                tt = res.tile([P, T], BF16, tag=f"tab{nm}")
                rrload(lambda e, tt=tt, t_=t_: e.dma_start(out=tt[:], in_=t_[:]))
                tabs[nm] = tt
            load_ht_q(1, 0, nc.scalar)
            load_ht_q(1, 1, nc.gpsimd)
            load_ht_q(1, 2, nc.scalar)
            load_ht_q(1, 3, nc.gpsimd)
            mask_sb = res.tile([P, max(n_masks, 1) * 128], BF16, tag="masks")
            nc.gpsimd.dma_start(out=mask_sb[:].rearrange("p (n j) -> p n j", j=128),
                                in_=masks[:].rearrange("n p j -> p n j"))
            nc.gpsimd.dma_start(out=wo_sb[:].rearrange("p (c j) -> p c j", c=HPC),
                                in_=wo[:].rearrange("(c p) j -> p c j", p=P))

            # persistent tensors
            qT = res.tile([P, HPC * T], BF16, tag="qT")          # [k, h*T + t]
            kT = res.tile([P, T], BF16, tag="kT")                # [k, t]
            v_sb = res.tile([P, NT * 144], BF16, tag="v")        # [s, j*144+k], col128=1
            sg = res.tile([P, NT * 512], BF16, tag="sg")         # [t, i*512+h*128+k]
            invk = res.tile([P, NT], FP32, tag="invk")           # [s-in-tile, j]
            ones1 = res.tile([P, 1], BF16, tag="ones1")          # [k,1] ones
            onesF = res.tile([P, P], BF16, tag="onesF")          # [k,128] ones
            epsb = res.tile([P, 1], FP32, tag="epsb")
            scr = res.tile([P, 512], BF16, tag="scr")
            nc.vector.memset(scr[:], 0.001)
            nc.vector.memset(ones1[:], 1.0)
            nc.vector.memset(onesF[:], 1.0)
            nc.vector.memset(epsb[:], EPS)
            nc.vector.memset(v_sb[:], 1.0)

            def ht_ap(c, dc):
                """[P, 512] rhs for d-chunk dc of 512-chunk c."""
                return ht_q[(c, dc // 4)][:, (dc % 4) * 512:(dc % 4) * 512 + 512]

            # warm the ACT function tables during the DMA-bound prologue so
            # later function switches never stall a dependent PE instruction
            warm = stage.tile([P, 8], FP32, tag="warm")
            nc.vector.memset(warm[:], 1.0)
            nc.scalar.activation(warm[:, 0:1], warm[:, 0:1], AF.Square)
            nc.scalar.activation(warm[:, 1:2], warm[:, 0:1], AF.Ln,
                                 scale=1.0, bias=epsb[:])
            nc.scalar.activation(warm[:, 2:3], warm[:, 0:1], AF.Exp)
            nc.scalar.activation(warm[:, 3:4], warm[:, 0:1], AF.Tanh)
            nc.scalar.copy(warm[:, 4:5], warm[:, 0:1])
            # PE warmup: ~9us of garbage matmuls during the DMA-queue boot
            # dead-time so HAM reaches full clock before real work arrives
            wups = [bigp.tile([P, 512], FP32, tag="big", name=f"wup{x}")
                    for x in range(2)]
            for x in range(44):
                nc.tensor.matmul(wups[x % 2][:], scr[:, 0:128], scr[:],
                                 start=True, stop=True)

            # ---------------- emission units ----------------
            qps_of = {}

            def emit_filler(n):
                for x in range(n):
                    nc.tensor.matmul(wups[x % 2][:], scr[:, 0:128], scr[:],
                                     start=True, stop=True)

            def emit_qT_mm(c, hs, fillers=0):
                """Transposed q proj matmuls for heads hs (dc-major). With 4
                heads the 4th accumulator borrows a big-pool bank (acc has 3).
                fillers>0 adds garbage warm-keeper matmuls per dc (region 0:
                DMA-bound, keeps HAM at full clock)."""
                for dc in range(NDC):
                    if fillers:
                        emit_filler(fillers)
                    for n_h, h in enumerate(hs):
                        if dc == 0:
                            pool = bigp if n_h == 3 else accp
                            tag = "big" if n_h == 3 else "acc"
                            qps_of[(c, h)] = pool.tile([P, 512], FP32, tag=tag,
                                                       name=f"qps{c}_{h}")
                        nc.tensor.matmul(
                            qps_of[(c, h)][:],
                            wq_sb[:, dc * 1024 + h * 128: dc * 1024 + (h + 1) * 128],
                            ht_ap(c, dc), start=(dc == 0), stop=(dc == NDC - 1))

            qn_of = {}

            def emit_qT_evict(c, h):
                """Single psum read: evict q_ps to bf16 SBUF (frees acc slot)."""
                q_ps = qps_of.pop((c, h))
                qn = stage.tile([P, 512], BF16, tag="qn", bufs=6)
                nc.scalar.copy(qn[:], q_ps[:])
                qn_of[(c, h)] = qn

            def emit_qT_post(c, h):
                """rms-norm (ones-matmul) + invq + rope -> qT."""
                qn = qn_of.pop((c, h))
                qsq = stage.tile([P, 512], BF16, tag="qsq", bufs=2)
                nc.scalar.activation(qsq[:], qn[:], AF.Square)
                ss_ps = bigp.tile([P, 512], FP32, tag="big")
                nc.tensor.matmul(ss_ps[:], onesF[:], qsq[:], start=True,
                                 stop=True)
                lnt = stage.tile([P, 512], FP32, tag="lnt", bufs=2)
                nc.scalar.activation(lnt[:], ss_ps[:], AF.Ln, scale=1.0 / K,
                                     bias=epsb[:])
                ivq = stage.tile([P, 512], BF16, tag="ivq", bufs=2)
                nc.scalar.activation(ivq[:], lnt[:], AF.Exp, scale=-0.5)
                # qi = qn * invq (gpsimd); rope on DVE from SBUF
                qi = stage.tile([P, 512], BF16, tag="qi", bufs=2)
                nc.gpsimd.tensor_tensor(qi[:], qn[:], ivq[:], ALU.mult)
                cqt = tabs["cq"][:, c * 512:(c + 1) * 512]
                sqt = tabs["sq"][:, c * 512:(c + 1) * 512]
                tmpc = stage.tile([P, 512], BF16, tag="tmpc", bufs=2)
                nc.vector.tensor_tensor(tmpc[:], qi[:], cqt, ALU.mult)
                # sq table rows are pre-rolled by 64 on the host so both
                # inputs of each half-product share a base partition
                tmps = stage.tile([P, 512], BF16, tag="tmps", bufs=2)
                nc.vector.tensor_tensor(tmps[0:64, :], qi[64:128, :],
                                        sqt[64:128, :], ALU.mult)
                nc.vector.tensor_tensor(tmps[64:128, :], qi[0:64, :],
                                        sqt[0:64, :], ALU.mult)
                nc.vector.tensor_tensor(
                    qT[:, h * T + c * 512: h * T + (c + 1) * 512],
                    tmpc[:], tmps[:], ALU.add)

            def emit_kTvT_mm(c):
                """k and v transposed proj, dc-major so consecutive matmuls
                alternate PSUM banks (same-bank back-to-back stalls ~44ns)."""
                k_ps = accp.tile([P, 512], FP32, tag="acc", name=f"kps{c}")
                vt_ps = accp.tile([P, 512], FP32, tag="acc", name=f"vtps{c}")
                for dc in range(NDC):
                    nc.tensor.matmul(k_ps[:],
                                     wk_sb[:, dc * 128:(dc + 1) * 128],
                                     ht_ap(c, dc), start=(dc == 0),
                                     stop=(dc == NDC - 1))
                    nc.tensor.matmul(vt_ps[:],
                                     wv_sb[:, dc * 128:(dc + 1) * 128],
                                     ht_ap(c, dc), start=(dc == 0),
                                     stop=(dc == NDC - 1))
                qps_of[(c, "k")] = k_ps
                vt = stage.tile([P, 512], BF16, tag="vt", bufs=2)
                nc.vector.tensor_copy(vt[:], vt_ps[:])
                for r in range(4):
                    j = c * 4 + r
                    nc.sync.dma_start(out=v_sb[:, j * 144:j * 144 + 128],
                                      in_=vt[:, r * 128:(r + 1) * 128],
                                      transpose=True)

            def emit_kT_evict(c):
                k_ps = qps_of.pop((c, "k"))
                kn = stage.tile([P, 512], BF16, tag="qn", bufs=6)
                nc.scalar.copy(kn[:], k_ps[:])
                qn_of[(c, "k")] = kn

            def emit_kT_post(c):
                kn = qn_of.pop((c, "k"))
                ksq = stage.tile([P, 512], BF16, tag="qsq", bufs=2)
                nc.scalar.activation(ksq[:], kn[:], AF.Square)
                ssk_ps = bigp.tile([P, 512], FP32, tag="big")
                for r in range(4):
                    nc.tensor.matmul(ssk_ps[:, r:r + 1],
                                     ksq[:, r * 128:(r + 1) * 128], ones1[:],
                                     start=True, stop=True)
                lnk = stage.tile([P, 4], FP32, tag="lnk")
                nc.scalar.activation(lnk[:], ssk_ps[:, 0:4], AF.Ln, scale=1.0 / K,
                                     bias=epsb[:])
                nc.scalar.activation(invk[:, c * 4:c * 4 + 4], lnk[:], AF.Exp,
                                     scale=-0.5)
                ckt = tabs["ck"][:, c * 512:(c + 1) * 512]
                skt = tabs["sk"][:, c * 512:(c + 1) * 512]
                tmpc = stage.tile([P, 512], BF16, tag="tmpc", bufs=2)
                nc.vector.tensor_tensor(tmpc[:], kn[:], ckt, ALU.mult)
                tmps = stage.tile([P, 512], BF16, tag="tmps", bufs=2)
                nc.vector.tensor_tensor(tmps[0:64, :], kn[64:128, :],
                                        skt[64:128, :], ALU.mult)
                nc.vector.tensor_tensor(tmps[64:128, :], kn[0:64, :],
                                        skt[0:64, :], ALU.mult)
                nc.vector.tensor_tensor(kT[:, c * 512:(c + 1) * 512], tmpc[:],
                                        tmps[:], ALU.add)

            def emit_gate(c, rs):
                """Gate proj (normal orient) + sigmoid -> sg; rs is a pair —
                dc-major so consecutive matmuls alternate PSUM banks."""
                gps = {}
                for dc in range(NDC):
                    for r in rs:
                        if dc == 0:
                            gps[r] = accp.tile([P, 512], FP32, tag="acc",
                                               name=f"gps{c}_{r}")
                        lhsT = ht_ap(c, dc)[:, r * 128:(r + 1) * 128]
                        nc.tensor.matmul(gps[r][:], lhsT,
                                         wq_sb[:, dc * 1024 + 512:dc * 1024 + 1024],
                                         start=(dc == 0), stop=(dc == NDC - 1))
                for r in rs:
                    i = c * 4 + r
                    sgc = sg[:, i * 512:(i + 1) * 512]
                    nc.scalar.activation(sgc, gps[r][:], AF.Tanh, scale=0.5)
                    nc.gpsimd.tensor_scalar(sgc, sgc, 0.5, 0.5, ALU.mult, ALU.add)

            # attention state per chunk
            pt_of = {}    # (h, j) -> (pt_tile, r0)
            ag_of = {}    # c -> ag tile
            agT_of = {}   # c -> agT tile

            def emit_qk(c, h):
                for (j, r0, r1) in runs[c]:
                    n = (r1 - r0 + 1) * 128
                    ptp = bigp.tile([P, 512], FP32, tag="big")
                    nc.tensor.matmul(
                        ptp[:, 0:n], kT[:, j * 128:(j + 1) * 128],
                        qT[:, h * T + c * 512 + r0 * 128:
                           h * T + c * 512 + r0 * 128 + n],
                        start=True, stop=True)
                    pt = ptpool.tile([P, 512], BF16, tag="pt")
                    nc.scalar.activation(pt[:, 0:n], ptp[:, 0:n], AF.Exp,
                                         scale=invk[:, j:j + 1])
                    for r in range(r0, r1 + 1):
                        i = c * 4 + r
                        if cls[i, j] == PARTIAL:
                            m = mask_idx[(i, j)]
                            sl = pt[:, (r - r0) * 128:(r - r0 + 1) * 128]
                            nc.gpsimd.tensor_tensor(
                                sl, sl, mask_sb[:, m * 128:(m + 1) * 128],
                                ALU.mult)
                    pt_of[(h, j)] = (pt, r0)

            def emit_pv(c, h, rs=range(4)):
                """PV + gating; r-streams interleaved in pairs so consecutive
                accumulating matmuls alternate PSUM banks."""
                if c not in ag_of:
                    ag_of[c] = agp.tile([P, 2048], BF16, tag="ag",
                                        name=f"ag{c}")
                ag = ag_of[c]
                rl = list(rs)
                for pair in (rl[x:x + 2] for x in range(0, len(rl), 2)):
                    pvps = {}
                    for r in pair:
                        pvps[r] = bigp.tile([P, 512], FP32, tag="big",
                                            name=f"pvp{c}_{h}_{r}")
                    nmax = max(len(pv[c * 4 + r]) for r in pair)
                    for nj in range(nmax):
                        for r in pair:
                            jl = pv[c * 4 + r]
                            if nj >= len(jl):
                                continue
                            j = jl[nj]
                            pt, r0 = pt_of[(h, j)]
                            nc.tensor.matmul(
                                pvps[r][:, 0:129],
                                pt[:, (r - r0) * 128:(r - r0 + 1) * 128],
                                v_sb[:, j * 144:j * 144 + 129],
                                start=(nj == 0), stop=(nj == len(jl) - 1))
                    for r in pair:
                        i = c * 4 + r
                        rd = stage.tile([P, 1], FP32, tag="rd", bufs=4)
                        nc.vector.reciprocal(rd[:], pvps[r][:, 128:129])
                        nc.vector.scalar_tensor_tensor(
                            ag[:, r * 512 + h * 128: r * 512 + (h + 1) * 128],
                            pvps[r][:, 0:128], rd[:],
                            sg[:, i * 512 + h * 128: i * 512 + (h + 1) * 128],
                            ALU.mult, ALU.mult)

            def emit_agT(c, rs=range(4)):
                """Transpose ag tiles (r,h) -> agT. h==0 tiles go through DVE
                StreamTranspose (batched); h 1-3 through sync-queue DMA
                transposes (that queue is otherwise light)."""
                ag = ag_of[c]
                if c not in agT_of:
                    agT_of[c] = agp.tile([P, 2048], BF16, tag="agT",
                                         name=f"agT{c}")
                agT = agT_of[c]
                agr = ag[:].rearrange("p (u f x) -> p u f x", u=4, f=4)
                agTr = agT[:].rearrange("p (u f x) -> p u f x", u=4, f=4)
                rl = list(rs)
                u0, u1 = rl[0], rl[-1] + 1
                for a in range(4):
                    for b in range(4):
                        nc.vector.transpose(
                            agTr[32 * a:32 * a + 32, u0:u1, 0,
                                 32 * b:32 * b + 32],
                            agr[32 * b:32 * b + 32, u0:u1, 0,
                                32 * a:32 * a + 32])
                for r in rl:
                    for h in range(1, 4):
                        u = r * 4 + h
                        nc.sync.dma_start(
                            out=agT[:, u * 128:(u + 1) * 128],
                            in_=ag[:, u * 128:(u + 1) * 128], transpose=True)

            def emit_oproj(c, r, act_only=False):
                """o_proj for t-tile i: d4-pairs interleaved h-major so
                consecutive matmuls alternate PSUM banks; each 512-col block
                stores out on its own rotating queue as soon as it's evicted.
                act_only routes all evictions to ACT (tail: DVE carries the
                agT stream-transposes on the critical path)."""
                agT = agT_of[c]
                i = c * 4 + r
                for d4p in (0, 2):
                    ops = {}
                    for d4 in (d4p, d4p + 1):
                        ops[d4] = bigp.tile([P, 512], FP32, tag="big",
                                            name=f"ops{c}_{r}_{d4}")
                    for h in range(HPC):
                        for d4 in (d4p, d4p + 1):
                            nc.tensor.matmul(
                                ops[d4][:],
                                agT[:, (r * 4 + h) * 128:(r * 4 + h + 1) * 128],
                                wo_sb[:, h * D + d4 * 512: h * D + (d4 + 1) * 512],
                                start=(h == 0), stop=(h == HPC - 1))
                    for d4 in (d4p, d4p + 1):
                        oo = oop.tile([P, 512], BF16, tag="oo", bufs=6)
                        if act_only or d4 % 2 == 0:
                            nc.scalar.copy(oo[:], ops[d4][:])
                        else:
                            nc.vector.tensor_copy(oo[:], ops[d4][:])
                        eng = (nc.sync, nc.gpsimd, nc.scalar)[(i * 4 + d4) % 3]
                        eng.dma_start(
                            out=out[i * 128:(i + 1) * 128,
                                    d4 * 512:(d4 + 1) * 512],
                            in_=oo[:])

            # ---------------- the weave ----------------
            for c in range(NC):
                pc = c - 1
                if c == 0:
                    # all 4 q heads dc-major (4 matmuls per wq slice matches
                    # the ~0.8us/slice striped delivery); kTvT after (wk/wv
                    # land last); evicts free acc slots early
                    emit_qT_mm(0, [0, 1, 2, 3], fillers=2)
                    emit_qT_evict(0, 0)
                    emit_qT_evict(0, 1)
                    emit_qT_evict(0, 2)
                    emit_qT_evict(0, 3)
                    emit_qT_post(0, 0)
                    emit_qT_post(0, 1)
                    emit_filler(8)
                    emit_kTvT_mm(0)
                    emit_kT_evict(0)
                    emit_qT_post(0, 2)
                    emit_qT_post(0, 3)
                    emit_kT_post(0)
                    emit_gate(0, [0, 1])
                    emit_gate(0, [2, 3])
                else:
                    emit_qT_mm(c, [0, 1])
                    emit_qk(pc, 0)
                    emit_qT_evict(c, 0)
                    emit_qT_evict(c, 1)
                    emit_qT_post(c, 0)
                    emit_pv(pc, 0)
                    emit_qk(pc, 1)
                    emit_qT_mm(c, [2, 3])
                    emit_qT_evict(c, 2)
                    emit_qT_evict(c, 3)
                    emit_qT_post(c, 1)
                    emit_pv(pc, 1)
                    emit_qk(pc, 2)
                    emit_qT_post(c, 2)
                    emit_kTvT_mm(c)
                    emit_kT_evict(c)
                    emit_pv(pc, 2)
                    emit_qk(pc, 3)
                    emit_qT_post(c, 3)
                    emit_kT_post(c)
                    emit_gate(c, [0, 1])
                    emit_pv(pc, 3)
                    emit_agT(pc)
                    emit_gate(c, [2, 3])
                    if c >= 2:
                        for r in range(4):
                            emit_oproj(c - 2, r)
                if c + 2 < NC:
                    load_ht_q(c + 2, 0, nc.scalar)
                    load_ht_q(c + 2, 1, nc.scalar)
                    load_ht_q(c + 2, 2, nc.scalar)
                    load_ht_q(c + 2, 3, nc.scalar)

            # tail: attention chunk 3 with o_proj chunks 2/3 as PE filler
            emit_qk(3, 0)
            emit_qk(3, 1)
            emit_oproj(2, 0)
            emit_pv(3, 0)
            emit_qk(3, 2)
            emit_oproj(2, 1)
            emit_pv(3, 1)
            emit_qk(3, 3)
            emit_oproj(2, 2)
            emit_pv(3, 2)
            emit_oproj(2, 3)
            emit_pv(3, 3, rs=[0])
            emit_agT(3, rs=[0])
            emit_pv(3, 3, rs=[1])
            emit_agT(3, rs=[1])
            emit_oproj(3, 0)
            emit_pv(3, 3, rs=[2])
            emit_agT(3, rs=[2])
            emit_oproj(3, 1)
            emit_pv(3, 3, rs=[3])
            emit_agT(3, rs=[3])
            emit_oproj(3, 2)
            emit_oproj(3, 3)

    split_multiwaits(nc)
    return nc


def _install_ntff_hook():
    """Best-effort NTFF profiling hook (axon containers); harmless if absent."""
    import contextlib, ctypes, types
    if "antenv.axon_hooks" in sys.modules:
        return
    lib = ctypes.CDLL("/opt/axon/libaxon_pjrt.so")
    if not hasattr(lib, "axon_start_nrt_profile"):
        raise RuntimeError("no profile symbols")
    lib.axon_start_nrt_profile.argtypes = [ctypes.POINTER(ctypes.c_int64), ctypes.c_size_t]
    lib.axon_start_nrt_profile.restype = ctypes.c_int64
    lib.axon_stop_nrt_profile.argtypes = [ctypes.c_char_p]
    lib.axon_stop_nrt_profile.restype = ctypes.c_int64

    @contextlib.contextmanager
    def _hook(output_dir, device_ids):
        import jax
        jax.devices()
        if device_ids:
            ids = (ctypes.c_int64 * len(device_ids))(*device_ids)
            rc = lib.axon_start_nrt_profile(ids, len(device_ids))
        else:
            rc = lib.axon_start_nrt_profile(None, 0)
        if rc != 0:
            raise RuntimeError(f"axon_start_nrt_profile rc={rc}")
        try:
            yield
        finally:
            lib.axon_stop_nrt_profile(str(output_dir).encode())

    store = {"h": _hook}
    mod = types.ModuleType("antenv.axon_hooks")
    mod.get_axon_ntff_profile_hook = lambda: store.get("h")
    mod.set_axon_ntff_profile_hook = lambda h: store.__setitem__("h", h)
    import antenv
    antenv.axon_hooks = mod
    sys.modules["antenv.axon_hooks"] = mod


def kernel(hidden, cos, sin, segment_ids, position_ids, Wq, Wk, Wv, Wo,
           q_norm_w, k_norm_w):
    hidden = np.asarray(hidden, np.float32)
    cos = np.asarray(cos, np.float32)
    sin = np.asarray(sin, np.float32)
    segment_ids = np.asarray(segment_ids)
    position_ids = np.asarray(position_ids)
    Wq = np.asarray(Wq, np.float32)
    Wk = np.asarray(Wk, np.float32)
    Wv = np.asarray(Wv, np.float32)
    Wo = np.asarray(Wo, np.float32)
    q_norm_w = np.asarray(q_norm_w, np.float32)
    k_norm_w = np.asarray(k_norm_w, np.float32)

    cls, mask_idx, masksT, runs, pv = _build_schedule(segment_ids, position_ids)
    n_masks = len(masksT[0])

    # RoPE tables with norm weights / scale / rotate-sign folded in, [K, T]
    rolled_q = np.roll(q_norm_w, -64)      # w[(k+64)%128]
    rolled_k = np.roll(k_norm_w, -64)
    sign = np.where(np.arange(K) < 64, -1.0, 1.0).astype(np.float32)
    in_maps = []
    for core in range(8):
        b, g = core // 4, core % 4
        # wq cols per core: [q_h0..q_h3 | gate_h0..gate_h3]
        wq_full = Wq[:, g * 4 * 256:(g + 1) * 4 * 256].reshape(D, 4, 2, K)
        wq_core = np.concatenate(
            [wq_full[:, :, 0, :].reshape(D, 512),
             wq_full[:, :, 1, :].reshape(D, 512)], axis=1)
        m = dict(
            hidT=np.ascontiguousarray(hidden[b].T).astype(BF),
            wq=wq_core.astype(BF),
            wk=Wk[:, g * K:(g + 1) * K].astype(BF),
            wv=Wv[:, g * K:(g + 1) * K].astype(BF),
            wo=Wo[g * 512:(g + 1) * 512, :].astype(BF),
            cq=np.ascontiguousarray((cos[b] * q_norm_w[None, :] * SCALE).T).astype(BF),
            sq=np.ascontiguousarray(np.roll(
                (sin[b] * rolled_q[None, :] * sign[None, :] * SCALE).T, 64,
                axis=0)).astype(BF),
            ck=np.ascontiguousarray((cos[b] * k_norm_w[None, :]).T).astype(BF),
            sk=np.ascontiguousarray(np.roll(
                (sin[b] * rolled_k[None, :] * sign[None, :]).T, 64,
                axis=0)).astype(BF),
            masks=(np.stack(masksT[b], 0) if n_masks
                   else np.zeros((1, 128, 128), np.float32)).astype(BF),
        )
        in_maps.append(m)

    nc = _build_program(n_masks, cls, mask_idx, runs, pv)
    res = None
    try:
        _install_ntff_hook()
        res = run_bass_kernel_spmd(nc, in_maps, list(range(8)), trace=True)
    except Exception:
        res = None
    if res is None:
        res = run_bass_kernel_spmd(nc, in_maps, list(range(8)))
    out = np.zeros((B, T, D), np.float32)
    for core in range(8):
        b = core // 4
        out[b] += res.results[core]["out"].astype(np.float32)
    kernel.last_results = res
    return out


if __name__ == "__main__":
    pass


# revision 3
# speedup vs baseline: 1.1796x; 1.1796x over previous
"""Trainium2 Bass kernel v2 for GQA attention (B=2,T=2048,D=2048,H=16,G=4,K=128)
with QK RMS-norm, RoPE, segment-aware causal masking, sigmoid gating, o_proj.

Sharding: 8 cores = (batch b, kv-group g); core c -> b=c//4, g=c%4.
Each core computes its 4 q-heads + 1 kv head end-to-end and a partial
o_proj ([T,D]); the host sums the 4 partials per batch (TP unshard).

v2: q/k are projected directly transposed (lhsT = weight cols, rhs = hidT),
so no DMA transposes; RMS-norm sums come from ones-matmuls; inv-rms(k)
folds into the exp's per-partition scale AP; inv-rms(q) is produced
pre-broadcast by an all-ones stationary matmul. QK is run-sliced to the
contiguous non-empty i-range. The attention-output transpose (agT, o_proj
lhsT) uses batched DVE StreamTranspose. Emission interleaves
proj(c) | attention(c-1) | o_proj(c-2) per 512-chunk to keep PE dense.
"""
import sys

sys.path.insert(0, "/opt/trn_rl_repo")

import numpy as np
import ml_dtypes

import concourse.bass as bass
import concourse.mybir as mybir
import concourse.tile as tile
from concourse.bass_utils import run_bass_kernel_spmd

FP32 = mybir.dt.float32
BF16 = mybir.dt.bfloat16
AF = mybir.ActivationFunctionType
ALU = mybir.AluOpType
BF = ml_dtypes.bfloat16

B, T, D = 2, 2048, 2048
H, G, K = 16, 4, 128
HPC = H // G              # q-heads per core = 4
EPS = 1e-6
SCALE = K ** -0.5
NT = T // 128             # 16 t-tiles
NC = 4                    # 512-chunks
NDC = D // 128            # 16 d-chunks
EMPTY, FULL, PARTIAL = 0, 1, 2


def split_multiwaits(nc):
    """This container's walrus accepts one sync-wait per instruction; hoist
    extras into standalone single-wait EventSemaphore instructions."""
    n = 0
    for fn in nc.m.functions:
        for bb in fn.blocks:
            out = []
            for ins in bb.instructions:
                si = ins.sync_info
                if si is not None and si.on_wait and len(si.on_wait) > 1:
                    waits = list(si.on_wait)
                    for w in waits[:-1]:
                        n += 1
                        out.append(mybir.InstEventSemaphore(
                            name=f"{ins.name}-w{n}", engine=ins.engine,
                            ins=[], outs=[],
                            sync_info=mybir.SyncInfo(on_wait=[w], on_update=[])))
                    ins.sync_info = mybir.SyncInfo(
                        on_wait=[waits[-1]], on_update=list(si.on_update or []))
                out.append(ins)
            bb.instructions = out
    return n


def _classify(allowed):
    cls = np.zeros((NT, NT), np.int32)
    a4 = allowed.reshape(NT, 128, NT, 128)
    any_ = a4.any(axis=(1, 3))
    all_ = a4.all(axis=(1, 3))
    cls[any_ & all_] = FULL
    cls[any_ & ~all_] = PARTIAL
    return cls


def _build_schedule(segment_ids, position_ids):
    """Union schedule across batches (SPMD: one program for all cores) plus
    per-batch mask tiles for partial (i,j)."""
    allowed = []
    for b in range(B):
        pos = position_ids[b].astype(np.int64)
        seg = segment_ids[b].astype(np.int64)
        al = (pos[:, None] >= pos[None, :]) & (seg[:, None] == seg[None, :])
        allowed.append(al)
    cls_b = [_classify(al) for al in allowed]
    cls = np.maximum(cls_b[0], cls_b[1])  # EMPTY only if both empty
    cls[(cls_b[0] == FULL) & (cls_b[1] == PARTIAL)] = PARTIAL
    cls[(cls_b[1] == FULL) & (cls_b[0] == PARTIAL)] = PARTIAL

    mask_idx = {}
    masksT = [[], []]   # per batch: list of [128s,128t] f32 (deduped pairs)
    seen = {}
    for i in range(NT):
        for j in range(NT):
            if cls[i, j] == PARTIAL:
                subs = [allowed[b][i * 128:(i + 1) * 128, j * 128:(j + 1) * 128].T
                        for b in range(B)]
                key = subs[0].tobytes() + subs[1].tobytes()
                if key not in seen:
                    seen[key] = len(masksT[0])
                    for b in range(B):
                        masksT[b].append(subs[b].astype(np.float32))
                mask_idx[(i, j)] = seen[key]

    # per chunk: list of (j, r0, r1): contiguous run of non-empty i = 4c+r
    runs = []
    for c in range(NC):
        lst = []
        for j in range(NT):
            rs = [r for r in range(4) if cls[c * 4 + r, j] != EMPTY]
            if rs:
                assert rs == list(range(rs[0], rs[-1] + 1)), (c, j, rs)
                lst.append((j, rs[0], rs[-1]))
        runs.append(lst)
    pv = {i: [j for j in range(NT) if cls[i, j] != EMPTY] for i in range(NT)}
    return cls, mask_idx, masksT, runs, pv


def _build_program(n_masks, cls, mask_idx, runs, pv):
    nc = bass.Bass()
    P = 128
    hidT = nc.declare_dram_parameter("hidT", [D, T], BF16, isOutput=False)
    # wq cols per dc-block: [q_h0|q_h1|q_h2|q_h3|gate_h0..h3 (512)]
    wq = nc.declare_dram_parameter("wq", [D, HPC * 2 * K], BF16, isOutput=False)
    wk = nc.declare_dram_parameter("wk", [D, K], BF16, isOutput=False)
    wv = nc.declare_dram_parameter("wv", [D, K], BF16, isOutput=False)
    wo = nc.declare_dram_parameter("wo", [HPC * K, D], BF16, isOutput=False)
    # transposed rope tables [K, T]; q-tables have SCALE folded in
    cq = nc.declare_dram_parameter("cq", [K, T], BF16, isOutput=False)
    sq_ = nc.declare_dram_parameter("sq", [K, T], BF16, isOutput=False)
    ck = nc.declare_dram_parameter("ck", [K, T], BF16, isOutput=False)
    sk = nc.declare_dram_parameter("sk", [K, T], BF16, isOutput=False)
    masks = nc.declare_dram_parameter("masks", [max(n_masks, 1), 128, 128], BF16,
                                      isOutput=False)
    out = nc.declare_dram_parameter("out", [T, D], BF16, isOutput=True)

    maxrun = max(len(r) for r in runs)
    with tile.TileContext(nc) as tc:
        with tc.tile_pool(name="res", bufs=1) as res, \
             tc.tile_pool(name="hid", bufs=8) as hid, \
             tc.tile_pool(name="acc", bufs=3, space="PSUM") as accp, \
             tc.tile_pool(name="big", bufs=5, space="PSUM") as bigp, \
             tc.tile_pool(name="stage", bufs=3) as stage, \
             tc.tile_pool(name="pt", bufs=maxrun + 2) as ptpool, \
             tc.tile_pool(name="agp", bufs=2) as agp, \
             tc.tile_pool(name="oop", bufs=2) as oop:

            # ---- resident loads, woven across queues so PE starts fast ----
            wq_sb = res.tile([P, NDC * 1024], BF16, tag="wq")
            wk_sb = res.tile([P, NDC * 128], BF16, tag="wk")
            wv_sb = res.tile([P, NDC * 128], BF16, tag="wv")
            wo_sb = res.tile([P, HPC * D], BF16, tag="wo")
            ht_q = {}   # (chunk, quarter) -> tile [P, 4dc * 512]
            hsrc = hidT[:].rearrange("(c p) t -> p c t", p=P)

            def load_ht_q(c, q, eng):
                t_ = hid.tile([P, 4 * 512], BF16, tag="ht")
                eng.dma_start(
                    out=t_[:].rearrange("p (a j) -> p a j", a=4),
                    in_=hsrc[:, 4 * q:4 * q + 4, c * 512:(c + 1) * 512])
                ht_q[(c, q)] = t_

            def load_wq(dc, eng):
                eng.dma_start(out=wq_sb[:, dc * 1024:(dc + 1) * 1024],
                              in_=wq[dc * P:(dc + 1) * P, :])

            # Prologue: stripe loads round-robin over the 3 DMA queues in
            # the exact order region-0's dc-major qT matmuls consume them
            # (~110 GB/s per queue, ~330 aggregate). kTvT runs after qT.
            rr = [nc.sync, nc.gpsimd, nc.scalar]
            rrn = [0]

            def rrload(fn):
                fn(rr[rrn[0] % 3])
                rrn[0] += 1

            rrload(lambda e: load_ht_q(0, 0, e))
            for dc in range(NDC):
                rrload(lambda e, dc=dc: load_wq(dc, e))
                if dc % 4 == 3 and dc < 12:
                    rrload(lambda e, q=dc // 4 + 1: load_ht_q(0, q, e))
            rrload(lambda e: e.dma_start(
                out=wk_sb[:].rearrange("p (c j) -> p c j", c=NDC),
                in_=wk[:].rearrange("(c p) j -> p c j", p=P)))
            rrload(lambda e: e.dma_start(
                out=wv_sb[:].rearrange("p (c j) -> p c j", c=NDC),
                in_=wv[:].rearrange("(c p) j -> p c j", p=P)))
            tabs = {}
            for nm, t_ in (("cq", cq), ("sq", sq_), ("ck", ck), ("sk", sk)):
                tt = res.tile([P, T], BF16, tag=f"tab{nm}")
                rrload(lambda e, tt=tt, t_=t_: e.dma_start(out=tt[:], in_=t_[:]))
                tabs[nm] = tt
            load_ht_q(1, 0, nc.scalar)
            load_ht_q(1, 1, nc.gpsimd)
            load_ht_q(1, 2, nc.scalar)
            load_ht_q(1, 3, nc.gpsimd)
            mask_sb = res.tile([P, max(n_masks, 1) * 128], BF16, tag="masks")
            nc.gpsimd.dma_start(out=mask_sb[:].rearrange("p (n j) -> p n j", j=128),
                                in_=masks[:].rearrange("n p j -> p n j"))
            nc.gpsimd.dma_start(out=wo_sb[:].rearrange("p (c j) -> p c j", c=HPC),
                                in_=wo[:].rearrange("(c p) j -> p c j", p=P))

            # persistent tensors
            qT = res.tile([P, HPC * T], BF16, tag="qT")          # [k, h*T + t]
            kT = res.tile([P, T], BF16, tag="kT")                # [k, t]
            v_sb = res.tile([P, NT * 144], BF16, tag="v")        # [s, j*144+k], col128=1
            sg = res.tile([P, NT * 512], BF16, tag="sg")         # [t, i*512+h*128+k]
            invk = res.tile([P, NT], FP32, tag="invk")           # [s-in-tile, j]
            ones1 = res.tile([P, 1], BF16, tag="ones1")          # [k,1] ones
            onesF = res.tile([P, P], BF16, tag="onesF")          # [k,128] ones
            epsb = res.tile([P, 1], FP32, tag="epsb")
            scr = res.tile([P, 512], BF16, tag="scr")
            nc.vector.memset(scr[:], 0.001)
            nc.vector.memset(ones1[:], 1.0)
            nc.vector.memset(onesF[:], 1.0)
            nc.vector.memset(epsb[:], EPS)
            nc.vector.memset(v_sb[:], 1.0)

            def ht_ap(c, dc):
                """[P, 512] rhs for d-chunk dc of 512-chunk c."""
                return ht_q[(c, dc // 4)][:, (dc % 4) * 512:(dc % 4) * 512 + 512]

            # warm the ACT function tables during the DMA-bound prologue so
            # later function switches never stall a dependent PE instruction
            warm = stage.tile([P, 8], FP32, tag="warm")
            nc.vector.memset(warm[:], 1.0)
            nc.scalar.activation(warm[:, 0:1], warm[:, 0:1], AF.Square)
            nc.scalar.activation(warm[:, 1:2], warm[:, 0:1], AF.Ln,
                                 scale=1.0, bias=epsb[:])
            nc.scalar.activation(warm[:, 2:3], warm[:, 0:1], AF.Exp)
            nc.scalar.activation(warm[:, 3:4], warm[:, 0:1], AF.Tanh)
            nc.scalar.copy(warm[:, 4:5], warm[:, 0:1])
            # PE warmup: ~9us of garbage matmuls during the DMA-queue boot
            # dead-time so HAM reaches full clock before real work arrives
            wups = [bigp.tile([P, 512], FP32, tag="big", name=f"wup{x}")
                    for x in range(2)]
            for x in range(44):
                nc.tensor.matmul(wups[x % 2][:], scr[:, 0:128], scr[:],
                                 start=True, stop=True)

            # ---------------- emission units ----------------
            qps_of = {}

            def emit_filler(n):
                for x in range(n):
                    nc.tensor.matmul(wups[x % 2][:], scr[:, 0:128], scr[:],
                                     start=True, stop=True)

            def emit_qT_mm(c, hs, fillers=0):
                """Transposed q proj matmuls for heads hs (dc-major). With 4
                heads the 4th accumulator borrows a big-pool bank (acc has 3).
                fillers>0 adds garbage warm-keeper matmuls per dc (region 0:
                DMA-bound, keeps HAM at full clock)."""
                for dc in range(NDC):
                    if fillers:
                        emit_filler(fillers)
                    for n_h, h in enumerate(hs):
                        if dc == 0:
                            pool = bigp if n_h == 3 else accp
                            tag = "big" if n_h == 3 else "acc"
                            qps_of[(c, h)] = pool.tile([P, 512], FP32, tag=tag,
                                                       name=f"qps{c}_{h}")
                        nc.tensor.matmul(
                            qps_of[(c, h)][:],
                            wq_sb[:, dc * 1024 + h * 128: dc * 1024 + (h + 1) * 128],
                            ht_ap(c, dc), start=(dc == 0), stop=(dc == NDC - 1))

            qn_of = {}

            def emit_qT_evict(c, h):
                """Single psum read: evict q_ps to bf16 SBUF (frees acc slot)."""
                q_ps = qps_of.pop((c, h))
                qn = stage.tile([P, 512], BF16, tag="qn", bufs=6)
                nc.scalar.copy(qn[:], q_ps[:])
                qn_of[(c, h)] = qn

            def emit_qT_post(c, h):
                """rms-norm (ones-matmul) + invq + rope -> qT."""
                qn = qn_of.pop((c, h))
                qsq = stage.tile([P, 512], BF16, tag="qsq", bufs=2)
                nc.scalar.activation(qsq[:], qn[:], AF.Square)
                ss_ps = bigp.tile([P, 512], FP32, tag="big")
                nc.tensor.matmul(ss_ps[:], onesF[:], qsq[:], start=True,
                                 stop=True)
                lnt = stage.tile([P, 512], FP32, tag="lnt", bufs=2)
                nc.scalar.activation(lnt[:], ss_ps[:], AF.Ln, scale=1.0 / K,
                                     bias=epsb[:])
                ivq = stage.tile([P, 512], BF16, tag="ivq", bufs=2)
                nc.scalar.activation(ivq[:], lnt[:], AF.Exp, scale=-0.5)
                # qi = qn * invq (gpsimd); rope on DVE from SBUF
                qi = stage.tile([P, 512], BF16, tag="qi", bufs=2)
                nc.gpsimd.tensor_tensor(qi[:], qn[:], ivq[:], ALU.mult)
                cqt = tabs["cq"][:, c * 512:(c + 1) * 512]
                sqt = tabs["sq"][:, c * 512:(c + 1) * 512]
                tmpc = stage.tile([P, 512], BF16, tag="tmpc", bufs=2)
                nc.vector.tensor_tensor(tmpc[:], qi[:], cqt, ALU.mult)
                # sq table rows are pre-rolled by 64 on the host so both
                # inputs of each half-product share a base partition
                tmps = stage.tile([P, 512], BF16, tag="tmps", bufs=2)
                nc.vector.tensor_tensor(tmps[0:64, :], qi[64:128, :],
                                        sqt[64:128, :], ALU.mult)
                nc.vector.tensor_tensor(tmps[64:128, :], qi[0:64, :],
                                        sqt[0:64, :], ALU.mult)
                nc.vector.tensor_tensor(
                    qT[:, h * T + c * 512: h * T + (c + 1) * 512],
                    tmpc[:], tmps[:], ALU.add)

            def emit_kTvT_mm(c):
                """k and v transposed proj, dc-major so consecutive matmuls
                alternate PSUM banks (same-bank back-to-back stalls ~44ns)."""
                k_ps = accp.tile([P, 512], FP32, tag="acc", name=f"kps{c}")
                vt_ps = accp.tile([P, 512], FP32, tag="acc", name=f"vtps{c}")
                for dc in range(NDC):
                    nc.tensor.matmul(k_ps[:],
                                     wk_sb[:, dc * 128:(dc + 1) * 128],
                                     ht_ap(c, dc), start=(dc == 0),
                                     stop=(dc == NDC - 1))
                    nc.tensor.matmul(vt_ps[:],
                                     wv_sb[:, dc * 128:(dc + 1) * 128],
                                     ht_ap(c, dc), start=(dc == 0),
                                     stop=(dc == NDC - 1))
                qps_of[(c, "k")] = k_ps
                vt = stage.tile([P, 512], BF16, tag="vt", bufs=2)
                nc.vector.tensor_copy(vt[:], vt_ps[:])
                for r in range(4):
                    j = c * 4 + r
                    nc.sync.dma_start(out=v_sb[:, j * 144:j * 144 + 128],
                                      in_=vt[:, r * 128:(r + 1) * 128],
                                      transpose=True)

            def emit_kT_evict(c):
                k_ps = qps_of.pop((c, "k"))
                kn = stage.tile([P, 512], BF16, tag="qn", bufs=6)
                nc.scalar.copy(kn[:], k_ps[:])
                qn_of[(c, "k")] = kn

            def emit_kT_post(c):
                kn = qn_of.pop((c, "k"))
                ksq = stage.tile([P, 512], BF16, tag="qsq", bufs=2)
                nc.scalar.activation(ksq[:], kn[:], AF.Square)
                ssk_ps = bigp.tile([P, 512], FP32, tag="big")
                for r in range(4):
                    nc.tensor.matmul(ssk_ps[:, r:r + 1],
                                     ksq[:, r * 128:(r + 1) * 128], ones1[:],
                                     start=True, stop=True)
                lnk = stage.tile([P, 4], FP32, tag="lnk")
                nc.scalar.activation(lnk[:], ssk_ps[:, 0:4], AF.Ln, scale=1.0 / K,
                                     bias=epsb[:])
                nc.scalar.activation(invk[:, c * 4:c * 4 + 4], lnk[:], AF.Exp,
                                     scale=-0.5)
                ckt = tabs["ck"][:, c * 512:(c + 1) * 512]
                skt = tabs["sk"][:, c * 512:(c + 1) * 512]
                tmpc = stage.tile([P, 512], BF16, tag="tmpc", bufs=2)
                nc.vector.tensor_tensor(tmpc[:], kn[:], ckt, ALU.mult)
                tmps = stage.tile([P, 512], BF16, tag="tmps", bufs=2)
                nc.vector.tensor_tensor(tmps[0:64, :], kn[64:128, :],
                                        skt[64:128, :], ALU.mult)
                nc.vector.tensor_tensor(tmps[64:128, :], kn[0:64, :],
                                        skt[0:64, :], ALU.mult)
                nc.vector.tensor_tensor(kT[:, c * 512:(c + 1) * 512], tmpc[:],
                                        tmps[:], ALU.add)

            def emit_gate(c, rs):
                """Gate proj (normal orient) + sigmoid -> sg; rs is a pair —
                dc-major so consecutive matmuls alternate PSUM banks."""
                gps = {}
                for dc in range(NDC):
                    for r in rs:
                        if dc == 0:
                            gps[r] = accp.tile([P, 512], FP32, tag="acc",
                                               name=f"gps{c}_{r}")
                        lhsT = ht_ap(c, dc)[:, r * 128:(r + 1) * 128]
                        nc.tensor.matmul(gps[r][:], lhsT,
                                         wq_sb[:, dc * 1024 + 512:dc * 1024 + 1024],
                                         start=(dc == 0), stop=(dc == NDC - 1))
                for r in rs:
                    i = c * 4 + r
                    sgc = sg[:, i * 512:(i + 1) * 512]
                    nc.scalar.activation(sgc, gps[r][:], AF.Sigmoid)

            # attention state per chunk
            pt_of = {}    # (h, j) -> (pt_tile, r0)
            ag_of = {}    # c -> ag tile
            agT_of = {}   # c -> agT tile

            def emit_qk(c, h):
                for (j, r0, r1) in runs[c]:
                    n = (r1 - r0 + 1) * 128
                    ptp = bigp.tile([P, 512], FP32, tag="big")
                    nc.tensor.matmul(
                        ptp[:, 0:n], kT[:, j * 128:(j + 1) * 128],
                        qT[:, h * T + c * 512 + r0 * 128:
                           h * T + c * 512 + r0 * 128 + n],
                        start=True, stop=True)
                    pt = ptpool.tile([P, 512], BF16, tag="pt")
                    nc.scalar.activation(pt[:, 0:n], ptp[:, 0:n], AF.Exp,
                                         scale=invk[:, j:j + 1])
                    for r in range(r0, r1 + 1):
                        i = c * 4 + r
                        if cls[i, j] == PARTIAL:
                            m = mask_idx[(i, j)]
                            sl = pt[:, (r - r0) * 128:(r - r0 + 1) * 128]
                            nc.gpsimd.tensor_tensor(
                                sl, sl, mask_sb[:, m * 128:(m + 1) * 128],
                                ALU.mult)
                    pt_of[(h, j)] = (pt, r0)

            def emit_pv(c, h, rs=range(4)):
                """PV + gating; r-streams interleaved in pairs so consecutive
                accumulating matmuls alternate PSUM banks."""
                if c not in ag_of:
                    ag_of[c] = agp.tile([P, 2048], BF16, tag="ag",
                                        name=f"ag{c}")
                ag = ag_of[c]
                rl = list(rs)
                for pair in (rl[x:x + 2] for x in range(0, len(rl), 2)):
                    pvps = {}
                    for r in pair:
                        pvps[r] = bigp.tile([P, 512], FP32, tag="big",
                                            name=f"pvp{c}_{h}_{r}")
                    nmax = max(len(pv[c * 4 + r]) for r in pair)
                    for nj in range(nmax):
                        for r in pair:
                            jl = pv[c * 4 + r]
                            if nj >= len(jl):
                                continue
                            j = jl[nj]
                            pt, r0 = pt_of[(h, j)]
                            nc.tensor.matmul(
                                pvps[r][:, 0:129],
                                pt[:, (r - r0) * 128:(r - r0 + 1) * 128],
                                v_sb[:, j * 144:j * 144 + 129],
                                start=(nj == 0), stop=(nj == len(jl) - 1))
                    for r in pair:
                        i = c * 4 + r
                        rd = stage.tile([P, 1], FP32, tag="rd", bufs=4)
                        nc.vector.reciprocal(rd[:], pvps[r][:, 128:129])
                        nc.vector.scalar_tensor_tensor(
                            ag[:, r * 512 + h * 128: r * 512 + (h + 1) * 128],
                            pvps[r][:, 0:128], rd[:],
                            sg[:, i * 512 + h * 128: i * 512 + (h + 1) * 128],
                            ALU.mult, ALU.mult)

            def emit_agT(c, rs=range(4)):
                """Transpose ag tiles (r,h) -> agT. h==0 tiles go through DVE
                StreamTranspose (batched); h 1-3 through sync-queue DMA
                transposes (that queue is otherwise light)."""
                ag = ag_of[c]
                if c not in agT_of:
                    agT_of[c] = agp.tile([P, 2048], BF16, tag="agT",
                                         name=f"agT{c}")
                agT = agT_of[c]
                agr = ag[:].rearrange("p (u f x) -> p u f x", u=4, f=4)
                agTr = agT[:].rearrange("p (u f x) -> p u f x", u=4, f=4)
                rl = list(rs)
                u0, u1 = rl[0], rl[-1] + 1
                for a in range(4):
                    for b in range(4):
                        nc.vector.transpose(
                            agTr[32 * a:32 * a + 32, u0:u1, 0,
                                 32 * b:32 * b + 32],
                            agr[32 * b:32 * b + 32, u0:u1, 0,
                                32 * a:32 * a + 32])
                for r in rl:
                    for h in range(1, 4):
                        u = r * 4 + h
                        nc.sync.dma_start(
                            out=agT[:, u * 128:(u + 1) * 128],
                            in_=ag[:, u * 128:(u + 1) * 128], transpose=True)

            def emit_oproj(c, r, act_only=False):
                """o_proj for t-tile i: d4-pairs interleaved h-major so
                consecutive matmuls alternate PSUM banks; each 512-col block
                stores out on its own rotating queue as soon as it's evicted.
                act_only routes all evictions to ACT (tail: DVE carries the
                agT stream-transposes on the critical path)."""
                agT = agT_of[c]
                i = c * 4 + r
                for d4p in (0, 2):
                    ops = {}
                    for d4 in (d4p, d4p + 1):
                        ops[d4] = bigp.tile([P, 512], FP32, tag="big",
                                            name=f"ops{c}_{r}_{d4}")
                    for h in range(HPC):
                        for d4 in (d4p, d4p + 1):
                            nc.tensor.matmul(
                                ops[d4][:],
                                agT[:, (r * 4 + h) * 128:(r * 4 + h + 1) * 128],
                                wo_sb[:, h * D + d4 * 512: h * D + (d4 + 1) * 512],
                                start=(h == 0), stop=(h == HPC - 1))
                    for d4 in (d4p, d4p + 1):
                        oo = oop.tile([P, 512], BF16, tag="oo", bufs=6)
                        if act_only or d4 % 2 == 0:
                            nc.scalar.copy(oo[:], ops[d4][:])
                        else:
                            nc.vector.tensor_copy(oo[:], ops[d4][:])
                        eng = (nc.sync, nc.gpsimd, nc.scalar)[(i * 4 + d4) % 3]
                        eng.dma_start(
                            out=out[i * 128:(i + 1) * 128,
                                    d4 * 512:(d4 + 1) * 512],
                            in_=oo[:])

            # ---------------- the weave ----------------
            for c in range(NC):
                pc = c - 1
                if c == 0:
                    # all 4 q heads dc-major (4 matmuls per wq slice matches
                    # the ~0.8us/slice striped delivery); kTvT after (wk/wv
                    # land last); evicts free acc slots early
                    emit_qT_mm(0, [0, 1, 2, 3], fillers=2)
                    emit_qT_evict(0, 0)
                    emit_qT_evict(0, 1)
                    emit_qT_evict(0, 2)
                    emit_qT_evict(0, 3)
                    emit_qT_post(0, 0)
                    emit_qT_post(0, 1)
                    emit_filler(8)
                    emit_kTvT_mm(0)
                    emit_kT_evict(0)
                    emit_qT_post(0, 2)
                    emit_qT_post(0, 3)
                    emit_kT_post(0)
                    emit_gate(0, [0, 1])
                    emit_gate(0, [2, 3])
                else:
                    emit_qT_mm(c, [0, 1])
                    emit_qk(pc, 0)
                    emit_qT_evict(c, 0)
                    emit_qT_evict(c, 1)
                    emit_qT_post(c, 0)
                    emit_pv(pc, 0)
                    emit_qk(pc, 1)
                    emit_qT_mm(c, [2, 3])
                    emit_qT_evict(c, 2)
                    emit_qT_evict(c, 3)
                    emit_qT_post(c, 1)
                    emit_pv(pc, 1)
                    emit_qk(pc, 2)
                    emit_qT_post(c, 2)
                    emit_kTvT_mm(c)
                    emit_kT_evict(c)
                    emit_pv(pc, 2)
                    emit_qk(pc, 3)
                    emit_qT_post(c, 3)
                    emit_kT_post(c)
                    emit_gate(c, [0, 1])
                    emit_pv(pc, 3, rs=[0, 1])
                    emit_agT(pc, rs=[0, 1])
                    emit_pv(pc, 3, rs=[2, 3])
                    emit_agT(pc, rs=[2, 3])
                    emit_gate(c, [2, 3])
                    if c >= 2:
                        for r in range(4):
                            emit_oproj(c - 2, r)
                if c + 2 < NC:
                    load_ht_q(c + 2, 0, nc.scalar)
                    load_ht_q(c + 2, 1, nc.scalar)
                    load_ht_q(c + 2, 2, nc.scalar)
                    load_ht_q(c + 2, 3, nc.scalar)

            # tail: o_proj(2) deferred until after the pv33/agT3 units so
            # its agT(2) DMA transposes have time to drain on the sync queue
            emit_qk(3, 0)
            emit_qk(3, 1)
            emit_pv(3, 0)
            emit_qk(3, 2)
            emit_pv(3, 1)
            emit_qk(3, 3)
            emit_oproj(2, 0)
            emit_pv(3, 2)
            emit_oproj(2, 1)
            emit_pv(3, 3, rs=[0])
            emit_agT(3, rs=[0])
            emit_oproj(2, 2)
            emit_pv(3, 3, rs=[1])
            emit_agT(3, rs=[1])
            emit_oproj(2, 3)
            emit_pv(3, 3, rs=[2])
            emit_agT(3, rs=[2])
            emit_oproj(3, 0)
            emit_pv(3, 3, rs=[3])
            emit_agT(3, rs=[3])
            emit_oproj(3, 1)
            emit_oproj(3, 2)
            emit_oproj(3, 3)

    split_multiwaits(nc)
    return nc


def _install_ntff_hook():
    """Best-effort NTFF profiling hook (axon containers); harmless if absent."""
    import contextlib, ctypes, types
    if "antenv.axon_hooks" in sys.modules:
        return
    lib = ctypes.CDLL("/opt/axon/libaxon_pjrt.so")
    if not hasattr(lib, "axon_start_nrt_profile"):
        raise RuntimeError("no profile symbols")
    lib.axon_start_nrt_profile.argtypes = [ctypes.POINTER(ctypes.c_int64), ctypes.c_size_t]
    lib.axon_start_nrt_profile.restype = ctypes.c_int64
    lib.axon_stop_nrt_profile.argtypes = [ctypes.c_char_p]
    lib.axon_stop_nrt_profile.restype = ctypes.c_int64

    @contextlib.contextmanager
    def _hook(output_dir, device_ids):
        import jax
        jax.devices()
        if device_ids:
            ids = (ctypes.c_int64 * len(device_ids))(*device_ids)
            rc = lib.axon_start_nrt_profile(ids, len(device_ids))
        else:
            rc = lib.axon_start_nrt_profile(None, 0)
        if rc != 0:
            raise RuntimeError(f"axon_start_nrt_profile rc={rc}")
        try:
            yield
        finally:
            lib.axon_stop_nrt_profile(str(output_dir).encode())

    store = {"h": _hook}
    mod = types.ModuleType("antenv.axon_hooks")
    mod.get_axon_ntff_profile_hook = lambda: store.get("h")
    mod.set_axon_ntff_profile_hook = lambda h: store.__setitem__("h", h)
    import antenv
    antenv.axon_hooks = mod
    sys.modules["antenv.axon_hooks"] = mod


def kernel(hidden, cos, sin, segment_ids, position_ids, Wq, Wk, Wv, Wo,
           q_norm_w, k_norm_w):
    hidden = np.asarray(hidden, np.float32)
    cos = np.asarray(cos, np.float32)
    sin = np.asarray(sin, np.float32)
    segment_ids = np.asarray(segment_ids)
    position_ids = np.asarray(position_ids)
    Wq = np.asarray(Wq, np.float32)
    Wk = np.asarray(Wk, np.float32)
    Wv = np.asarray(Wv, np.float32)
    Wo = np.asarray(Wo, np.float32)
    q_norm_w = np.asarray(q_norm_w, np.float32)
    k_norm_w = np.asarray(k_norm_w, np.float32)

    cls, mask_idx, masksT, runs, pv = _build_schedule(segment_ids, position_ids)
    n_masks = len(masksT[0])

    # RoPE tables with norm weights / scale / rotate-sign folded in, [K, T]
    rolled_q = np.roll(q_norm_w, -64)      # w[(k+64)%128]
    rolled_k = np.roll(k_norm_w, -64)
    sign = np.where(np.arange(K) < 64, -1.0, 1.0).astype(np.float32)
    in_maps = []
    for core in range(8):
        b, g = core // 4, core % 4
        # wq cols per core: [q_h0..q_h3 | gate_h0..gate_h3]
        wq_full = Wq[:, g * 4 * 256:(g + 1) * 4 * 256].reshape(D, 4, 2, K)
        wq_core = np.concatenate(
            [wq_full[:, :, 0, :].reshape(D, 512),
             wq_full[:, :, 1, :].reshape(D, 512)], axis=1)
        m = dict(
            hidT=np.ascontiguousarray(hidden[b].T).astype(BF),
            wq=wq_core.astype(BF),
            wk=Wk[:, g * K:(g + 1) * K].astype(BF),
            wv=Wv[:, g * K:(g + 1) * K].astype(BF),
            wo=Wo[g * 512:(g + 1) * 512, :].astype(BF),
            cq=np.ascontiguousarray((cos[b] * q_norm_w[None, :] * SCALE).T).astype(BF),
            sq=np.ascontiguousarray(np.roll(
                (sin[b] * rolled_q[None, :] * sign[None, :] * SCALE).T, 64,
                axis=0)).astype(BF),
            ck=np.ascontiguousarray((cos[b] * k_norm_w[None, :]).T).astype(BF),
            sk=np.ascontiguousarray(np.roll(
                (sin[b] * rolled_k[None, :] * sign[None, :]).T, 64,
                axis=0)).astype(BF),
            masks=(np.stack(masksT[b], 0) if n_masks
                   else np.zeros((1, 128, 128), np.float32)).astype(BF),
        )
        in_maps.append(m)

    nc = _build_program(n_masks, cls, mask_idx, runs, pv)
    res = None
    try:
        _install_ntff_hook()
        res = run_bass_kernel_spmd(nc, in_maps, list(range(8)), trace=True)
    except Exception:
        res = None
    if res is None:
        res = run_bass_kernel_spmd(nc, in_maps, list(range(8)))
    out = np.zeros((B, T, D), np.float32)
    for core in range(8):
        b = core // 4
        out[b] += res.results[core]["out"].astype(np.float32)
    kernel.last_results = res
    return out


if __name__ == "__main__":
    pass
